# revision 4
# baseline (speedup 1.0000x reference)
"""GAT 2-layer kernel for Trainium2, 8 NeuronCores.

Strategy (graph/data parallel, dst-sharded):
 - Host: sort edges by dst, pack per-core / per-dst-tile chunk streams
   (128 edges per chunk), fold attention vectors into the weight matrix so a
   single matmul produces per-node rows [h | a_src | a_dst].
 - Device, per layer: build T = x @ Wc (node feature table, bf16, in HBM),
   then for each dst-tile of 128 nodes: indirect-DMA gather T[src] rows,
   compute per-edge w = exp(leakyrelu(a_src+a_dst)), and aggregate
   numerator+denominator with a selection-matrix matmul into PSUM
   (out[d] = sum_e S[e,d] * [w*h[src] | w]).  Softmax normalization, bias,
   activation happen per dst-tile; no scatter is ever needed.
 - Two launches (layer1, layer2); host concatenates layer1 shards (the
   "all-to-all halo exchange" of the sharding hint).
"""

import numpy as np
import ml_dtypes
from contextlib import ExitStack

import concourse.bass as bass
import concourse.tile as tile
from concourse import bacc, mybir
from concourse.bass import ts, ds
from concourse.bass_utils import run_bass_kernel_spmd

PERF = {}  # filled with per-layer exec_time_ns when BASS_TRACE is enabled

BF16 = mybir.dt.bfloat16
F32 = mybir.dt.float32
I32 = mybir.dt.int32
NPBF16 = ml_dtypes.bfloat16

P = 128
NCORES = 8
N = 50000
E = 1600000
TPC = 49                      # dst tiles per core
G = NCORES * TPC              # 392 global tiles
NPAD = G * P                  # 50176 padded node count
NEG_SLOPE = 0.2


def _prep_edges(edge_index):
    """Sort edges by dst; build per-core [P, NCH] streams with per-tile
    chunk padding shared across cores (SPMD static shapes)."""
    src = edge_index[0].astype(np.int64)
    dst = edge_index[1].astype(np.int64)
    order = np.argsort(dst, kind="stable")
    srcs = src[order].astype(np.int32)
    dsts = dst[order]

    tile_of_edge = dsts >> 7                       # [E]
    counts = np.bincount(tile_of_edge, minlength=G)
    cnt2 = counts.reshape(NCORES, TPC)
    CH = np.maximum((cnt2 + P - 1) // P, 1).max(axis=0).astype(np.int64)  # [TPC]
    cumCH = np.concatenate([[0], np.cumsum(CH)]).astype(np.int64)
    NCH = int(cumCH[-1])

    src_arr = np.zeros((NCORES, P, NCH), np.int32)
    dsti_arr = np.zeros((NCORES, P, NCH), np.int32)
    dstf_arr = np.full((NCORES, P, NCH), -1.0, np.float32)

    tile_starts = np.concatenate([[0], np.cumsum(counts)])
    rank = np.arange(E, dtype=np.int64) - tile_starts[tile_of_edge]
    core_of_edge = tile_of_edge // TPC
    ltile = tile_of_edge % TPC
    col = cumCH[ltile] + (rank >> 7)
    part = rank & 127

    src_arr[core_of_edge, part, col] = srcs
    dsti_arr[core_of_edge, part, col] = dsts.astype(np.int32)
    dstf_arr[core_of_edge, part, col] = (dsts & 127).astype(np.float32)
    return src_arr, dsti_arr, dstf_arr, [int(c) for c in CH], cumCH


def _build_layer_program(KIN, F_G, F_D, CH_list, cumCH, NCH, layer):
    """One SPMD Bass program for one GAT layer.

    KIN: input feature dim (256 / 64); F_G: gathered row width (64+F_D_src),
    F_D: heads (8 / 1). Layer 1 outputs bf16 elu(...); layer 2 outputs f32
    log_softmax rows.
    """
    F_H = 64
    WCW = F_G + F_D          # built table row width (h | a_src | a_dst)
    RW = F_H + F_D           # matmul rhs / psum width (msg | w)
    KT = (KIN + P - 1) // P  # K tiles for the build matmul
    KP = min(KIN, P)         # partition size of build lhsT
    CHmax = max(CH_list)
    out_dt = BF16 if layer == 1 else F32

    nc = bacc.Bacc("TRN2", target_bir_lowering=False, debug=False,
                   num_devices=NCORES)

    xT_in = nc.dram_tensor("xT", [KIN, NPAD], BF16, kind="ExternalInput").ap()
    wc_in = nc.dram_tensor("wc", [KIN, WCW], BF16, kind="ExternalInput").ap()
    src_in = nc.dram_tensor("srcs", [P, NCH], I32, kind="ExternalInput").ap()
    dsti_in = nc.dram_tensor("dsti", [P, NCH], I32, kind="ExternalInput").ap()
    dstf_in = nc.dram_tensor("dstf", [P, NCH], F32, kind="ExternalInput").ap()
    bias_in = nc.dram_tensor("bias", [1, F_H], F32, kind="ExternalInput").ap()
    out_dram = nc.dram_tensor("out", [TPC * P, F_H], out_dt,
                              kind="ExternalOutput").ap()

    with tile.TileContext(nc) as tc, ExitStack() as ctx:
        cpool = ctx.enter_context(tc.tile_pool(name="const", bufs=1))
        dpool = ctx.enter_context(tc.tile_pool(name="dram", bufs=1,
                                               space=bass.MemorySpace.DRAM))
        bpool = ctx.enter_context(tc.tile_pool(name="bld", bufs=3))
        epool = ctx.enter_context(tc.tile_pool(name="edge", bufs=2))
        opool = ctx.enter_context(tc.tile_pool(name="post", bufs=2))
        pps = ctx.enter_context(tc.tile_pool(name="psb", bufs=2,
                                             space=bass.MemorySpace.PSUM))
        ppe = ctx.enter_context(tc.tile_pool(name="pse", bufs=2,
                                             space=bass.MemorySpace.PSUM))

        # ---- constants ----
        wc_sb = cpool.tile([KP, KT, WCW], BF16)
        for kt in range(KT):
            nc.sync.dma_start(wc_sb[:, kt, :], wc_in[kt * KP:(kt + 1) * KP, :])
        bias_sb = cpool.tile([P, F_H], F32)
        nc.sync.dma_start(bias_sb[:], bias_in.to_broadcast((P, F_H)))
        iota_i = cpool.tile([P, CHmax, P], I32)
        nc.gpsimd.iota(iota_i[:], pattern=[[0, CHmax], [1, P]],
                       channel_multiplier=0)
        iota_f = cpool.tile([P, CHmax, P], F32)
        nc.vector.tensor_copy(iota_f[:], iota_i[:])

        # ---- phase 1: build T = [h | a_src] and Tb = [a_dst] for all nodes
        T_dram = dpool.tile([NPAD, F_G], BF16)
        Tb_dram = dpool.tile([NPAD, F_D], BF16)
        for t in range(G):
            xt = bpool.tile([KP, KT, P], BF16)
            for kt in range(KT):
                nc.sync.dma_start(xt[:, kt, :],
                                  xT_in[kt * KP:(kt + 1) * KP, ts(t, P)])
            psB = pps.tile([P, WCW], F32)
            for kt in range(KT):
                nc.tensor.matmul(psB[:], xt[:, kt, :], wc_sb[:, kt, :],
                                 start=(kt == 0), stop=(kt == KT - 1))
            tcast = bpool.tile([P, WCW], BF16)
            nc.vector.tensor_copy(tcast[:], psB[:])
            nc.sync.dma_start(T_dram[ts(t, P), :], tcast[:, 0:F_G])
            nc.sync.dma_start(Tb_dram[ts(t, P), :], tcast[:, F_G:WCW])

        # ---- phase 2: per dst-tile edge aggregation ----
        for t in range(TPC):
            CH = CH_list[t]
            c0 = int(cumCH[t])
            src_t = epool.tile([P, CHmax], I32)
            nc.sync.dma_start(src_t[:, 0:CH], src_in[:, ds(c0, CH)])
            dsti_t = epool.tile([P, CHmax], I32)
            nc.sync.dma_start(dsti_t[:, 0:CH], dsti_in[:, ds(c0, CH)])
            dstf_t = epool.tile([P, CHmax, 1], F32)
            nc.sync.dma_start(dstf_t[:, 0:CH, 0], dstf_in[:, ds(c0, CH)])

            G_t = epool.tile([P, CHmax, F_G], BF16)
            D_t = epool.tile([P, CHmax, F_D], BF16)
            for c in range(CH):
                nc.gpsimd.indirect_dma_start(
                    out=G_t[:, c, :], out_offset=None, in_=T_dram[:],
                    in_offset=bass.IndirectOffsetOnAxis(
                        ap=src_t[:, c:c + 1], axis=0))
                nc.gpsimd.indirect_dma_start(
                    out=D_t[:, c, :], out_offset=None, in_=Tb_dram[:],
                    in_offset=bass.IndirectOffsetOnAxis(
                        ap=dsti_t[:, c:c + 1], axis=0))

            # edge logits -> w = exp(leakyrelu(a_src + a_dst))
            L_t = epool.tile([P, CHmax, F_D], F32)
            nc.vector.tensor_add(L_t[:, 0:CH, :], G_t[:, 0:CH, F_H:F_G],
                                 D_t[:, 0:CH, :])
            L2_t = epool.tile([P, CHmax, F_D], F32)
            nc.scalar.activation(L2_t[:, 0:CH, :], L_t[:, 0:CH, :],
                                 mybir.ActivationFunctionType.Lrelu,
                                 alpha=NEG_SLOPE)
            rhs_t = epool.tile([P, CHmax, RW], BF16)
            nc.scalar.activation(rhs_t[:, 0:CH, F_H:RW], L2_t[:, 0:CH, :],
                                 mybir.ActivationFunctionType.Exp)
            # msg = w * h[src]  (replicate each head's w across its channels)
            CW = F_H // F_D
            for h in range(F_D):
                nc.vector.tensor_mul(
                    rhs_t[:, 0:CH, h * CW:(h + 1) * CW],
                    G_t[:, 0:CH, h * CW:(h + 1) * CW],
                    rhs_t[:, 0:CH, F_H + h:F_H + h + 1].to_broadcast(
                        (P, CH, CW)))
            # selection matrix S[e, d] = (dst_local[e] == d)
            S_t = epool.tile([P, CHmax, P], BF16)
            nc.vector.tensor_tensor(
                S_t[:, 0:CH, :],
                dstf_t[:, 0:CH, :].to_broadcast((P, CH, P)),
                iota_f[:, 0:CH, :], op=mybir.AluOpType.is_equal)

            psE = ppe.tile([P, RW], F32)
            for c in range(CH):
                nc.tensor.matmul(psE[:], S_t[:, c, :], rhs_t[:, c, :],
                                 start=(c == 0), stop=(c == CH - 1))

            # ---- postprocess this dst-tile ----
            if layer == 1:
                den = opool.tile([P, F_D], F32)
                nc.vector.tensor_scalar_add(den[:], psE[:, F_H:RW], 1e-16)
                rec = opool.tile([P, F_D], F32)
                nc.vector.reciprocal(rec[:], den[:])
                o1 = opool.tile([P, F_H], F32)
                for h in range(F_D):
                    nc.vector.tensor_mul(
                        o1[:, h * CW:(h + 1) * CW],
                        psE[:, h * CW:(h + 1) * CW],
                        rec[:, h:h + 1].to_broadcast((P, CW)))
                nc.vector.tensor_add(o1[:], o1[:], bias_sb[:])
                # elu(x) = max(x,0) + exp(min(x,0)) - 1
                mn = opool.tile([P, F_H], F32)
                nc.vector.tensor_scalar_min(mn[:], o1[:], 0.0)
                em = opool.tile([P, F_H], F32)
                nc.scalar.activation(em[:], mn[:],
                                     mybir.ActivationFunctionType.Exp)
                mx = opool.tile([P, F_H], F32)
                nc.vector.tensor_scalar_max(mx[:], o1[:], 0.0)
                s1 = opool.tile([P, F_H], F32)
                nc.vector.tensor_add(s1[:], mx[:], em[:])
                ob = opool.tile([P, F_H], BF16)
                nc.vector.tensor_scalar_add(ob[:], s1[:], -1.0)
                nc.sync.dma_start(out_dram[ts(t, P), :], ob[:])
            else:
                den = opool.tile([P, 1], F32)
                nc.vector.tensor_scalar_add(den[:], psE[:, F_H:RW], 1e-16)
                rec = opool.tile([P, 1], F32)
                nc.vector.reciprocal(rec[:], den[:])
                o2 = opool.tile([P, F_H], F32)
                nc.vector.tensor_mul(
                    o2[:], psE[:, 0:F_H], rec[:].to_broadcast((P, F_H)))
                nc.vector.tensor_add(o2[:], o2[:], bias_sb[:])
                rm = opool.tile([P, 1], F32)
                nc.vector.tensor_reduce(rm[:], o2[:], mybir.AxisListType.X,
                                        mybir.AluOpType.max)
                z = opool.tile([P, F_H], F32)
                nc.vector.tensor_tensor(z[:], o2[:],
                                        rm[:].to_broadcast((P, F_H)),
                                        op=mybir.AluOpType.subtract)
                e_t = opool.tile([P, F_H], F32)
                s_t = opool.tile([P, 1], F32)
                nc.scalar.activation(e_t[:], z[:],
                                     mybir.ActivationFunctionType.Exp,
                                     accum_out=s_t[:])
                ls = opool.tile([P, 1], F32)
                nc.scalar.activation(ls[:], s_t[:],
                                     mybir.ActivationFunctionType.Ln)
                of = opool.tile([P, F_H], F32)
                nc.vector.tensor_tensor(of[:], z[:],
                                        ls[:].to_broadcast((P, F_H)),
                                        op=mybir.AluOpType.subtract)
                nc.sync.dma_start(out_dram[ts(t, P), :], of[:])

    nc.compile()
    return nc


def _fold_weights1(W1, att_src1, att_dst1):
    A1s = np.zeros((64, 8), np.float32)
    A1s[np.arange(64), np.arange(64) // 8] = att_src1.reshape(64)
    A1d = np.zeros((64, 8), np.float32)
    A1d[np.arange(64), np.arange(64) // 8] = att_dst1.reshape(64)
    return np.concatenate([W1, W1 @ A1s, W1 @ A1d], axis=1)  # [256, 80]


def kernel(x, edge_index, W1, att_src1, att_dst1, bias1,
           W2, att_src2, att_dst2, bias2):
    src_arr, dsti_arr, dstf_arr, CH_list, cumCH = _prep_edges(edge_index)
    NCH = int(cumCH[-1])

    # ---------- layer 1 ----------
    Wc1 = _fold_weights1(W1, att_src1, att_dst1).astype(NPBF16)
    xT = np.zeros((256, NPAD), NPBF16)
    xT[:, :N] = x.T.astype(NPBF16)

    nc1 = _build_layer_program(256, 72, 8, CH_list, cumCH, NCH, layer=1)
    in_maps = [{
        "xT": xT, "wc": Wc1,
        "srcs": np.ascontiguousarray(src_arr[k]),
        "dsti": np.ascontiguousarray(dsti_arr[k]),
        "dstf": np.ascontiguousarray(dstf_arr[k]),
        "bias": bias1.astype(np.float32).reshape(1, 64),
    } for k in range(NCORES)]
    res1 = run_bass_kernel_spmd(nc1, in_maps, core_ids=list(range(NCORES)))
    PERF["layer1_ns"] = res1.exec_time_ns
    PERF["layer1_profile"] = res1.profile_json
    x2 = np.concatenate([res1.results[k]["out"] for k in range(NCORES)],
                        axis=0)  # [NPAD, 64] bf16

    # ---------- layer 2 ----------
    Wc2 = np.concatenate(
        [W2, W2 @ att_src2.T, W2 @ att_dst2.T], axis=1).astype(NPBF16)
    x2T = np.ascontiguousarray(x2.T)  # [64, NPAD] bf16

    nc2 = _build_layer_program(64, 65, 1, CH_list, cumCH, NCH, layer=2)
    in_maps2 = [{
        "xT": x2T, "wc": Wc2,
        "srcs": np.ascontiguousarray(src_arr[k]),
        "dsti": np.ascontiguousarray(dsti_arr[k]),
        "dstf": np.ascontiguousarray(dstf_arr[k]),
        "bias": bias2.astype(np.float32).reshape(1, 64),
    } for k in range(NCORES)]
    res2 = run_bass_kernel_spmd(nc2, in_maps2, core_ids=list(range(NCORES)))
    PERF["layer2_ns"] = res2.exec_time_ns
    PERF["layer2_profile"] = res2.profile_json
    out = np.concatenate([res2.results[k]["out"] for k in range(NCORES)],
                         axis=0)  # [NPAD, 64] f32
    return out[:N].astype(np.float32)



# revision 8
# speedup vs baseline: 1.7001x; 1.7001x over previous
"""GAT 2-layer kernel for Trainium2, 8 NeuronCores — dst-major dma_gather design.

Strategy (graph/data parallel, dst-sharded):
 - Host: for each core and each dst-tile of 128 nodes, lay edges out
   dst-major: the edge for destination node (tile, d) occupies partition
   row d.  Each row holds that node's incoming edges in consecutive chunk
   columns; empty slots point at a dummy table row whose a_src = -300 so
   its exp() weight underflows to 0 and h = 0 contributes nothing.
 - Device, per layer: build node table rows [h | a_src | junk] (256B bf16
   rows, split into a lo table (first 255 tiles) and hi table so indices
   fit in int16) plus an a_dst side table; per dst-tile, ONE dma_gather
   per table batch-gathers all edge rows, a [128,1] indirect DMA fetches
   the tile's a_dst rows, and the whole per-edge compute is elementwise:
   w = exp(leakyrelu(a_src + a_dst)), rhs = [w*h | w], then a single
   free-axis reduction over chunk columns yields numerator|denominator.
   Softmax normalization, bias, activation per dst-tile; no scatter.
 - Two launches (layer1, layer2); host concatenates layer1 shards (the
   all-to-all halo exchange of the sharding hint).
"""

import numpy as np
import ml_dtypes
from contextlib import ExitStack

import concourse.bass as bass
import concourse.tile as tile
from concourse import bacc, mybir
from concourse.bass import ts, ds
from concourse.bass_utils import run_bass_kernel_spmd

PERF = {}  # filled with per-layer exec_time_ns when BASS_TRACE is enabled

BF16 = mybir.dt.bfloat16
F32 = mybir.dt.float32
I32 = mybir.dt.int32
I16 = mybir.dt.int16
NPBF16 = ml_dtypes.bfloat16

P = 128
NCORES = 8
N = 50000
E = 1600000
TPC = 49                      # dst tiles per core
G = NCORES * TPC              # 392 global tiles
NPAD = G * P                  # 50176 padded node count
NEG_SLOPE = 0.2
TW = 128                      # table row width (bf16) = 256B

LO_T = 255                    # build tiles in the lo table
LO = LO_T * P                 # 32640 first hi node
NLOROWS = LO + 1              # +1 dummy row
NHIROWS = (NPAD - LO) + 1     # 17537
DUMLO = LO                    # dummy row index in lo table
DUMHI = NPAD - LO             # dummy row index in hi table


def _prep_edges(edge_index):
    """Dst-major slot grids with lo/hi split; int16 gather indices in the
    16-partition wrapped layout (replicated across the 8 q7 cores)."""
    src = edge_index[0].astype(np.int64)
    dst = edge_index[1].astype(np.int64)
    hi = src >= LO
    key = dst * 2 + hi
    order = np.argsort(key, kind="stable")
    s_src = src[order]
    s_key = key[order]
    cnts = np.bincount(s_key, minlength=2 * NPAD)
    starts = np.concatenate([[0], np.cumsum(cnts)])
    rank = np.arange(E, dtype=np.int64) - starts[s_key]
    s_dst = dst[order]
    s_hi = hi[order]
    gt = s_dst >> 7
    d = s_dst & 127
    core = gt // TPC
    lt = gt % TPC

    deg_lo = cnts[0::2].reshape(G, P)
    deg_hi = cnts[1::2].reshape(G, P)
    CHlo = np.maximum(deg_lo.max(1).reshape(NCORES, TPC).max(0), 1)
    CHhi = np.maximum(deg_hi.max(1).reshape(NCORES, TPC).max(0), 1)
    cumlo = np.concatenate([[0], np.cumsum(CHlo)]).astype(np.int64)
    cumhi = np.concatenate([[0], np.cumsum(CHhi)]).astype(np.int64)
    NLOC, NHIC = int(cumlo[-1]), int(cumhi[-1])

    idxlo = np.full((NCORES, 16, 8 * NLOC), DUMLO, np.int16)
    idxhi = np.full((NCORES, 16, 8 * NHIC), DUMHI, np.int16)
    m = ~s_hi
    j = rank[m] * P + d[m]
    idxlo[core[m], j & 15, 8 * cumlo[lt[m]] + (j >> 4)] = \
        s_src[m].astype(np.int16)
    m = s_hi
    j = rank[m] * P + d[m]
    idxhi[core[m], j & 15, 8 * cumhi[lt[m]] + (j >> 4)] = \
        (s_src[m] - LO).astype(np.int16)
    idxlo = np.ascontiguousarray(np.tile(idxlo, (1, 8, 1)))
    idxhi = np.ascontiguousarray(np.tile(idxhi, (1, 8, 1)))

    dstnode = np.empty((NCORES, P, TPC), np.int32)
    for k in range(NCORES):
        dstnode[k] = ((k * TPC + np.arange(TPC)) * P)[None, :] \
            + np.arange(P)[:, None]

    return (idxlo, idxhi, dstnode,
            [int(c) for c in CHlo], [int(c) for c in CHhi], cumlo, cumhi)


def _build_layer_program(KIN, F_D, CHlo, CHhi, cumlo, cumhi, layer):
    """One SPMD Bass program for one GAT layer.

    KIN: input feature dim (256 / 64); F_D: heads (8 / 1).  Layer 1
    outputs bf16 elu(...); layer 2 outputs f32 log_softmax rows.
    """
    F_H = 64
    F_G = F_H + F_D          # gathered row payload (h | a_src)
    WCW = F_G + F_D          # built row width (h | a_src | a_dst)
    RW = F_H + F_D           # reduce width (msg | w)
    CW = F_H // F_D          # channels per head
    KT = (KIN + P - 1) // P
    KP = min(KIN, P)
    NLOC, NHIC = int(cumlo[-1]), int(cumhi[-1])
    out_dt = BF16 if layer == 1 else F32

    nc = bacc.Bacc("TRN2", target_bir_lowering=False, debug=False,
                   num_devices=NCORES)

    xT_in = nc.dram_tensor("xT", [KIN, NPAD], BF16, kind="ExternalInput").ap()
    wc_in = nc.dram_tensor("wc", [KIN, WCW], BF16, kind="ExternalInput").ap()
    dum_in = nc.dram_tensor("dummyrow", [1, TW], BF16,
                            kind="ExternalInput").ap()
    il_in = nc.dram_tensor("idxlo", [P, 8 * NLOC], I16,
                           kind="ExternalInput").ap()
    ih_in = nc.dram_tensor("idxhi", [P, 8 * NHIC], I16,
                           kind="ExternalInput").ap()
    dn_in = nc.dram_tensor("dstnode", [P, TPC], I32,
                           kind="ExternalInput").ap()
    bias_in = nc.dram_tensor("bias", [1, F_H], F32, kind="ExternalInput").ap()
    out_dram = nc.dram_tensor("out", [TPC * P, F_H], out_dt,
                              kind="ExternalOutput").ap()

    with tile.TileContext(nc) as tc, ExitStack() as ctx:
        cpool = ctx.enter_context(tc.tile_pool(name="const", bufs=1))
        dpool = ctx.enter_context(tc.tile_pool(name="dram", bufs=1,
                                               space=bass.MemorySpace.DRAM))
        bpool = ctx.enter_context(tc.tile_pool(name="bld", bufs=3))
        epool = ctx.enter_context(tc.tile_pool(name="edge", bufs=2))
        opool = ctx.enter_context(tc.tile_pool(name="post", bufs=2))
        pps = ctx.enter_context(tc.tile_pool(name="psb", bufs=2,
                                             space=bass.MemorySpace.PSUM))

        # ---- constants ----
        wc_sb = cpool.tile([KP, KT, WCW], BF16)
        for kt in range(KT):
            nc.sync.dma_start(wc_sb[:, kt, :], wc_in[kt * KP:(kt + 1) * KP, :])
        bias_sb = cpool.tile([P, F_H], F32)
        nc.sync.dma_start(bias_sb[:], bias_in.to_broadcast((P, F_H)))
        il_sb = cpool.tile([P, 8 * NLOC], I16)
        nc.sync.dma_start(il_sb[:], il_in[:, :])
        ih_sb = cpool.tile([P, 8 * NHIC], I16)
        nc.sync.dma_start(ih_sb[:], ih_in[:, :])
        dn_sb = cpool.tile([P, TPC], I32)
        nc.sync.dma_start(dn_sb[:], dn_in[:, :])
        dum_sb = cpool.tile([1, TW], BF16)
        nc.sync.dma_start(dum_sb[:], dum_in[:, :])

        # ---- phase 1: build node tables ----
        T_lo = dpool.tile([NLOROWS, TW], BF16)
        T_hi = dpool.tile([NHIROWS, TW], BF16)
        Tb = dpool.tile([NPAD, F_D], BF16)
        nc.sync.dma_start(T_lo[DUMLO:DUMLO + 1, :], dum_sb[:])
        nc.sync.dma_start(T_hi[DUMHI:DUMHI + 1, :], dum_sb[:])

        for g8 in range(G // 8):
            t0 = g8 * 8
            xt8 = bpool.tile([KP, KT, 8 * P], BF16)
            nc.scalar.dma_start(
                xt8[:],
                xT_in[:, ds(t0 * P, 8 * P)].rearrange("(k p) f -> p k f",
                                                      p=KP))
            tc8 = bpool.tile([P, 8, F_G], BF16)
            tb8 = bpool.tile([P, 8, F_D], BF16)
            for j in range(8):
                psB = pps.tile([P, WCW], F32)
                for kt in range(KT):
                    nc.tensor.matmul(psB[:], xt8[:, kt, ds(j * P, P)],
                                     wc_sb[:, kt, :],
                                     start=(kt == 0), stop=(kt == KT - 1))
                nc.vector.tensor_copy(tc8[:, j, :], psB[:, 0:F_G])
                nc.vector.tensor_copy(tb8[:, j, :], psB[:, F_G:WCW])
            # T write (lo/hi split at tile LO_T)
            if t0 + 8 <= LO_T:
                nc.gpsimd.dma_start(
                    T_lo[ds(t0 * P, 8 * P), 0:F_G].rearrange(
                        "(j p) f -> p j f", p=P), tc8[:])
            elif t0 >= LO_T:
                nc.gpsimd.dma_start(
                    T_hi[ds((t0 - LO_T) * P, 8 * P), 0:F_G].rearrange(
                        "(j p) f -> p j f", p=P), tc8[:])
            else:
                nl = LO_T - t0
                nc.gpsimd.dma_start(
                    T_lo[ds(t0 * P, nl * P), 0:F_G].rearrange(
                        "(j p) f -> p j f", p=P), tc8[:, 0:nl, :])
                nc.gpsimd.dma_start(
                    T_hi[ds(0, (8 - nl) * P), 0:F_G].rearrange(
                        "(j p) f -> p j f", p=P), tc8[:, nl:8, :])
            nc.sync.dma_start(
                Tb[ds(t0 * P, 8 * P), :].rearrange("(j p) f -> p j f", p=P),
                tb8[:])

        # ---- phase 2: per dst-tile edge aggregation ----
        for t in range(TPC):
            CHl, CHh = CHlo[t], CHhi[t]
            CHt = CHl + CHh
            adst = epool.tile([P, 1, F_D], BF16)
            nc.gpsimd.indirect_dma_start(
                out=adst[:, 0, :], out_offset=None, in_=Tb[:],
                in_offset=bass.IndirectOffsetOnAxis(
                    ap=dn_sb[:, t:t + 1], axis=0))
            G_t = epool.tile([P, CHt, TW], BF16)
            nc.gpsimd.dma_gather(
                G_t[:, 0:CHl, :], T_lo[:, :],
                il_sb[:, ds(8 * int(cumlo[t]), 8 * CHl)],
                P * CHl, P * CHl, TW, single_packet=False)
            nc.gpsimd.dma_gather(
                G_t[:, CHl:CHt, :], T_hi[:, :],
                ih_sb[:, ds(8 * int(cumhi[t]), 8 * CHh)],
                P * CHh, P * CHh, TW, single_packet=False)

            # per-edge weights: w = exp(leakyrelu(a_src + a_dst))
            L = epool.tile([P, CHt, F_D], F32)
            nc.vector.tensor_add(L[:], G_t[:, :, F_H:F_G],
                                 adst.to_broadcast((P, CHt, F_D)))
            Lr = epool.tile([P, CHt, F_D], F32)
            nc.scalar.activation(Lr[:], L[:],
                                 mybir.ActivationFunctionType.Lrelu,
                                 alpha=NEG_SLOPE)
            rhs = epool.tile([P, CHt, RW], BF16)
            nc.scalar.activation(rhs[:, :, F_H:RW], Lr[:],
                                 mybir.ActivationFunctionType.Exp)
            for h in range(F_D):
                nc.vector.tensor_mul(
                    rhs[:, :, h * CW:(h + 1) * CW],
                    G_t[:, :, h * CW:(h + 1) * CW],
                    rhs[:, :, F_H + h:F_H + h + 1].to_broadcast(
                        (P, CHt, CW)))
            # reduce over chunk columns: [P, CHt, RW] -> [P, RW]
            acc = opool.tile([P, RW, 1], F32)
            nc.vector.tensor_reduce(acc[:], rhs[:].rearrange("p c f -> p f c"),
                                    mybir.AxisListType.X, mybir.AluOpType.add)

            # ---- postprocess this dst-tile ----
            if layer == 1:
                den = opool.tile([P, F_D], F32)
                nc.vector.tensor_scalar_add(den[:], acc[:, F_H:RW, 0], 1e-16)
                rec = opool.tile([P, F_D], F32)
                nc.vector.reciprocal(rec[:], den[:])
                o1 = opool.tile([P, F_H], F32)
                for h in range(F_D):
                    nc.vector.tensor_mul(
                        o1[:, h * CW:(h + 1) * CW],
                        acc[:, h * CW:(h + 1) * CW, 0],
                        rec[:, h:h + 1].to_broadcast((P, CW)))
                nc.vector.tensor_add(o1[:], o1[:], bias_sb[:])
                # elu(x) = max(x,0) + exp(min(x,0)) - 1
                mn = opool.tile([P, F_H], F32)
                nc.vector.tensor_scalar_min(mn[:], o1[:], 0.0)
                em = opool.tile([P, F_H], F32)
                nc.scalar.activation(em[:], mn[:],
                                     mybir.ActivationFunctionType.Exp)
                mx = opool.tile([P, F_H], F32)
                nc.vector.tensor_scalar_max(mx[:], o1[:], 0.0)
                s1 = opool.tile([P, F_H], F32)
                nc.vector.tensor_add(s1[:], mx[:], em[:])
                ob = opool.tile([P, F_H], BF16)
                nc.vector.tensor_scalar_add(ob[:], s1[:], -1.0)
                nc.sync.dma_start(out_dram[ts(t, P), :], ob[:])
            else:
                den = opool.tile([P, 1], F32)
                nc.vector.tensor_scalar_add(den[:], acc[:, F_H:RW, 0], 1e-16)
                rec = opool.tile([P, 1], F32)
                nc.vector.reciprocal(rec[:], den[:])
                o2 = opool.tile([P, F_H], F32)
                nc.vector.tensor_mul(
                    o2[:], acc[:, 0:F_H, 0], rec[:].to_broadcast((P, F_H)))
                nc.vector.tensor_add(o2[:], o2[:], bias_sb[:])
                rm = opool.tile([P, 1], F32)
                nc.vector.tensor_reduce(rm[:], o2[:], mybir.AxisListType.X,
                                        mybir.AluOpType.max)
                z = opool.tile([P, F_H], F32)
                nc.vector.tensor_tensor(z[:], o2[:],
                                        rm[:].to_broadcast((P, F_H)),
                                        op=mybir.AluOpType.subtract)
                e_t = opool.tile([P, F_H], F32)
                s_t = opool.tile([P, 1], F32)
                nc.scalar.activation(e_t[:], z[:],
                                     mybir.ActivationFunctionType.Exp,
                                     accum_out=s_t[:])
                ls = opool.tile([P, 1], F32)
                nc.scalar.activation(ls[:], s_t[:],
                                     mybir.ActivationFunctionType.Ln)
                of = opool.tile([P, F_H], F32)
                nc.vector.tensor_tensor(of[:], z[:],
                                        ls[:].to_broadcast((P, F_H)),
                                        op=mybir.AluOpType.subtract)
                nc.sync.dma_start(out_dram[ts(t, P), :], of[:])

    nc.compile()
    return nc


def _fold_weights1(W1, att_src1, att_dst1):
    A1s = np.zeros((64, 8), np.float32)
    A1s[np.arange(64), np.arange(64) // 8] = att_src1.reshape(64)
    A1d = np.zeros((64, 8), np.float32)
    A1d[np.arange(64), np.arange(64) // 8] = att_dst1.reshape(64)
    return np.concatenate([W1, W1 @ A1s, W1 @ A1d], axis=1)  # [256, 80]


def _dummy_row():
    row = np.zeros((1, TW), np.float32)
    row[0, 64:72] = -300.0
    return row.astype(NPBF16)


def kernel(x, edge_index, W1, att_src1, att_dst1, bias1,
           W2, att_src2, att_dst2, bias2):
    idxlo, idxhi, dstnode, CHlo, CHhi, cumlo, cumhi = _prep_edges(edge_index)
    dummyrow = _dummy_row()

    # ---------- layer 1 ----------
    Wc1 = _fold_weights1(W1, att_src1, att_dst1).astype(NPBF16)
    xT = np.zeros((256, NPAD), NPBF16)
    xT[:, :N] = x.T.astype(NPBF16)

    nc1 = _build_layer_program(256, 8, CHlo, CHhi, cumlo, cumhi, layer=1)
    in_maps = [{
        "xT": xT, "wc": Wc1, "dummyrow": dummyrow,
        "idxlo": idxlo[k], "idxhi": idxhi[k],
        "dstnode": np.ascontiguousarray(dstnode[k]),
        "bias": bias1.astype(np.float32).reshape(1, 64),
    } for k in range(NCORES)]
    res1 = run_bass_kernel_spmd(nc1, in_maps, core_ids=list(range(NCORES)))
    PERF["layer1_ns"] = res1.exec_time_ns
    PERF["layer1_profile"] = res1.profile_json
    x2 = np.concatenate([res1.results[k]["out"] for k in range(NCORES)],
                        axis=0)  # [NPAD, 64] bf16

    # ---------- layer 2 ----------
    Wc2 = np.concatenate(
        [W2, W2 @ att_src2.T, W2 @ att_dst2.T], axis=1).astype(NPBF16)
    x2T = np.ascontiguousarray(x2.T)  # [64, NPAD] bf16

    nc2 = _build_layer_program(64, 1, CHlo, CHhi, cumlo, cumhi, layer=2)
    in_maps2 = [{
        "xT": x2T, "wc": Wc2, "dummyrow": dummyrow,
        "idxlo": idxlo[k], "idxhi": idxhi[k],
        "dstnode": np.ascontiguousarray(dstnode[k]),
        "bias": bias2.astype(np.float32).reshape(1, 64),
    } for k in range(NCORES)]
    res2 = run_bass_kernel_spmd(nc2, in_maps2, core_ids=list(range(NCORES)))
    PERF["layer2_ns"] = res2.exec_time_ns
    PERF["layer2_profile"] = res2.profile_json
    out = np.concatenate([res2.results[k]["out"] for k in range(NCORES)],
                         axis=0)  # [NPAD, 64] f32
    return out[:N].astype(np.float32)


# revision 9
# speedup vs baseline: 3.1248x; 1.8380x over previous
"""GAT 2-layer kernel for Trainium2, 8 NeuronCores.

Dense-packed dst-sharded design:
 - Host: sort edges by destination tile; pack each (core, tile)'s edges
   densely into 128-row chunks (lo/hi src split so gather indices fit in
   int16).  Precompute per-chunk one-hot selection matrices: S  (edge ->
   dst-local, used by the PE to aggregate messages straight into PSUM)
   and S2 (its transpose, used by the PE to broadcast per-dst a_dst to
   the edge slots).  No scatter and no on-chip index arithmetic.
 - Device, per layer: phase 1 builds node table rows [h | a_src | junk]
   (256B bf16 rows, lo/hi tables) + a_dst side table by matmul; phase 2
   per dst-tile: batch dma_gather of all edge rows, per-edge
   w = exp(leakyrelu(a_src + a_dst)) elementwise, S-matmul accumulates
   [w*h | w] into PSUM, then softmax-normalize + bias + activation.
 - Two launches (layer1, layer2); host concatenates layer1 shards (the
   all-to-all halo exchange of the sharding hint).
"""

import numpy as np
import ml_dtypes
from contextlib import ExitStack

import concourse.bass as bass
import concourse.tile as tile
from concourse import bacc, mybir
from concourse.bass import ts, ds
from concourse.bass_utils import run_bass_kernel_spmd

PERF = {}  # filled with per-layer exec_time_ns when BASS_TRACE is enabled

BF16 = mybir.dt.bfloat16
F32 = mybir.dt.float32
I32 = mybir.dt.int32
I16 = mybir.dt.int16
NPBF16 = ml_dtypes.bfloat16

P = 128
NCORES = 8
N = 50000
E = 1600000
TPC = 49                      # dst tiles per core
G = NCORES * TPC              # 392 global tiles
NPAD = G * P                  # 50176 padded node count
NEG_SLOPE = 0.2
TW = 128                      # table row width (bf16) = 256B

LO_T = 255                    # build tiles in the lo table
LO = LO_T * P                 # 32640 first hi node
NLOROWS = LO + 1              # +1 dummy row
NHIROWS = (NPAD - LO) + 1     # 17537
DUMLO = LO                    # dummy row index in lo table
DUMHI = NPAD - LO             # dummy row index in hi table


def _prep_edges(edge_index):
    """Dense per-(core,tile) chunk packing with lo/hi src split, int16
    gather indices (16-partition wrap, replicated across q7 cores), and
    per-chunk one-hot S / S2 selection matrices."""
    src = edge_index[0].astype(np.int64)
    dst = edge_index[1].astype(np.int64)
    hi = src >= LO
    gt = dst >> 7
    key = gt * 2 + hi
    order = np.argsort(key, kind="stable")
    s_src = src[order]
    s_key = key[order]
    cnts = np.bincount(s_key, minlength=2 * G)
    starts = np.concatenate([[0], np.cumsum(cnts)])
    rank = np.arange(E, dtype=np.int64) - starts[s_key]
    s_d = (dst[order] & 127)
    s_hi = hi[order]
    s_gt = gt[order]
    core = s_gt // TPC
    lt = s_gt % TPC

    nlo = cnts[0::2].reshape(NCORES, TPC)
    nhi = cnts[1::2].reshape(NCORES, TPC)
    CHlo = np.maximum((nlo + P - 1) // P, 1).max(0)
    CHhi = np.maximum((nhi + P - 1) // P, 1).max(0)
    CH = CHlo + CHhi
    cumlo = np.concatenate([[0], np.cumsum(CHlo)]).astype(np.int64)
    cumhi = np.concatenate([[0], np.cumsum(CHhi)]).astype(np.int64)
    cumCH = np.concatenate([[0], np.cumsum(CH)]).astype(np.int64)
    NLOC, NHIC = int(cumlo[-1]), int(cumhi[-1])
    NCH = int(cumCH[-1])

    idxlo = np.full((NCORES, 16, 8 * NLOC), DUMLO, np.int16)
    idxhi = np.full((NCORES, 16, 8 * NHIC), DUMHI, np.int16)
    m = ~s_hi
    r = rank[m]
    idxlo[core[m], r & 15, 8 * cumlo[lt[m]] + (r >> 4)] = \
        s_src[m].astype(np.int16)
    m = s_hi
    r = rank[m]
    idxhi[core[m], r & 15, 8 * cumhi[lt[m]] + (r >> 4)] = \
        (s_src[m] - LO).astype(np.int16)
    idxlo = np.ascontiguousarray(np.tile(idxlo, (1, 8, 1)))
    idxhi = np.ascontiguousarray(np.tile(idxhi, (1, 8, 1)))

    # one-hot S (edge-slot partition -> dst-local column) and S2 (transpose)
    S_arr = np.zeros((NCORES, P, NCH * P), NPBF16)
    S2_arr = np.zeros((NCORES, P, NCH * P), NPBF16)
    # chunk index within the tile: lo chunks first, then hi chunks
    c_in_tile = np.where(s_hi, CHlo[lt] + (rank >> 7), rank >> 7)
    colbase = (cumCH[lt] + c_in_tile) * P
    p_slot = rank & 127
    S_arr[core, p_slot, colbase + s_d] = 1
    S2_arr[core, s_d, colbase + p_slot] = 1

    dstnode = np.empty((NCORES, P, TPC), np.int32)
    for k in range(NCORES):
        dstnode[k] = ((k * TPC + np.arange(TPC)) * P)[None, :] \
            + np.arange(P)[:, None]

    return (idxlo, idxhi, S_arr, S2_arr, dstnode,
            [int(c) for c in CHlo], [int(c) for c in CHhi],
            cumlo, cumhi, cumCH)


def _build_layer_program(KIN, F_D, CHlo, CHhi, cumlo, cumhi, cumCH, layer):
    """One SPMD Bass program for one GAT layer.

    KIN: input feature dim (256 / 64); F_D: heads (8 / 1).  Layer 1
    outputs bf16 elu(...); layer 2 outputs f32 log_softmax rows.
    """
    F_H = 64
    F_G = F_H + F_D          # gathered row payload (h | a_src)
    WCW = F_G + F_D          # built row width (h | a_src | a_dst)
    RW = F_H + F_D           # psum width (msg | w)
    CW = F_H // F_D          # channels per head
    KT = (KIN + P - 1) // P
    KP = min(KIN, P)
    NLOC, NHIC = int(cumlo[-1]), int(cumhi[-1])
    NCH = int(cumCH[-1])
    out_dt = BF16 if layer == 1 else F32

    nc = bacc.Bacc("TRN2", target_bir_lowering=False, debug=False,
                   num_devices=NCORES)

    xT_in = nc.dram_tensor("xT", [KIN, NPAD], BF16, kind="ExternalInput").ap()
    wc_in = nc.dram_tensor("wc", [KIN, WCW], BF16, kind="ExternalInput").ap()
    dum_in = nc.dram_tensor("dummyrow", [1, TW], BF16,
                            kind="ExternalInput").ap()
    il_in = nc.dram_tensor("idxlo", [P, 8 * NLOC], I16,
                           kind="ExternalInput").ap()
    ih_in = nc.dram_tensor("idxhi", [P, 8 * NHIC], I16,
                           kind="ExternalInput").ap()
    S_in = nc.dram_tensor("S", [P, NCH * P], BF16, kind="ExternalInput").ap()
    S2_in = nc.dram_tensor("S2", [P, NCH * P], BF16, kind="ExternalInput").ap()
    dn_in = nc.dram_tensor("dstnode", [P, TPC], I32,
                           kind="ExternalInput").ap()
    bias_in = nc.dram_tensor("bias", [1, F_H], F32, kind="ExternalInput").ap()
    out_dram = nc.dram_tensor("out", [TPC * P, F_H], out_dt,
                              kind="ExternalOutput").ap()

    with tile.TileContext(nc) as tc, ExitStack() as ctx:
        cpool = ctx.enter_context(tc.tile_pool(name="const", bufs=1))
        dpool = ctx.enter_context(tc.tile_pool(name="dram", bufs=1,
                                               space=bass.MemorySpace.DRAM))
        bpool = ctx.enter_context(tc.tile_pool(name="bld", bufs=3))
        spool = ctx.enter_context(tc.tile_pool(name="sel", bufs=3))
        epool = ctx.enter_context(tc.tile_pool(name="edge", bufs=3))
        opool = ctx.enter_context(tc.tile_pool(name="post", bufs=2))
        pps = ctx.enter_context(tc.tile_pool(name="psb", bufs=2,
                                             space=bass.MemorySpace.PSUM))
        ppl = ctx.enter_context(tc.tile_pool(name="psl", bufs=2,
                                             space=bass.MemorySpace.PSUM))
        ppe = ctx.enter_context(tc.tile_pool(name="pse", bufs=2,
                                             space=bass.MemorySpace.PSUM))

        # ---- constants ----
        wc_sb = cpool.tile([KP, KT, WCW], BF16)
        for kt in range(KT):
            nc.sync.dma_start(wc_sb[:, kt, :], wc_in[kt * KP:(kt + 1) * KP, :])
        bias_sb = cpool.tile([P, F_H], F32)
        nc.sync.dma_start(bias_sb[:], bias_in.to_broadcast((P, F_H)))
        il_sb = cpool.tile([P, 8 * NLOC], I16)
        nc.sync.dma_start(il_sb[:], il_in[:, :])
        ih_sb = cpool.tile([P, 8 * NHIC], I16)
        nc.sync.dma_start(ih_sb[:], ih_in[:, :])
        dn_sb = cpool.tile([P, TPC], I32)
        nc.sync.dma_start(dn_sb[:], dn_in[:, :])
        dum_sb = cpool.tile([1, TW], BF16)
        nc.sync.dma_start(dum_sb[:], dum_in[:, :])

        # ---- phase 1: build node tables ----
        T_lo = dpool.tile([NLOROWS, TW], BF16)
        T_hi = dpool.tile([NHIROWS, TW], BF16)
        Tb = dpool.tile([NPAD, F_D], BF16)
        nc.sync.dma_start(T_lo[DUMLO:DUMLO + 1, :], dum_sb[:])
        nc.sync.dma_start(T_hi[DUMHI:DUMHI + 1, :], dum_sb[:])

        for g8 in range(G // 8):
            t0 = g8 * 8
            xt8 = bpool.tile([KP, KT, 8 * P], BF16)
            nc.scalar.dma_start(
                xt8[:],
                xT_in[:, ds(t0 * P, 8 * P)].rearrange("(k p) f -> p k f",
                                                      p=KP))
            tc8 = bpool.tile([P, 8, F_G], BF16)
            tb8 = bpool.tile([P, 8, F_D], BF16)
            for j in range(8):
                psB = pps.tile([P, WCW], F32)
                for kt in range(KT):
                    nc.tensor.matmul(psB[:], xt8[:, kt, ds(j * P, P)],
                                     wc_sb[:, kt, :],
                                     start=(kt == 0), stop=(kt == KT - 1))
                nc.vector.tensor_copy(tc8[:, j, :], psB[:, 0:F_G])
                nc.vector.tensor_copy(tb8[:, j, :], psB[:, F_G:WCW])
            # T write (lo/hi split at tile LO_T)
            if t0 + 8 <= LO_T:
                nc.gpsimd.dma_start(
                    T_lo[ds(t0 * P, 8 * P), 0:F_G].rearrange(
                        "(j p) f -> p j f", p=P), tc8[:])
            elif t0 >= LO_T:
                nc.gpsimd.dma_start(
                    T_hi[ds((t0 - LO_T) * P, 8 * P), 0:F_G].rearrange(
                        "(j p) f -> p j f", p=P), tc8[:])
            else:
                nl = LO_T - t0
                nc.gpsimd.dma_start(
                    T_lo[ds(t0 * P, nl * P), 0:F_G].rearrange(
                        "(j p) f -> p j f", p=P), tc8[:, 0:nl, :])
                nc.gpsimd.dma_start(
                    T_hi[ds(0, (8 - nl) * P), 0:F_G].rearrange(
                        "(j p) f -> p j f", p=P), tc8[:, nl:8, :])
            nc.sync.dma_start(
                Tb[ds(t0 * P, 8 * P), :].rearrange("(j p) f -> p j f", p=P),
                tb8[:])

        # ---- phase 2: per dst-tile edge aggregation ----
        for t in range(TPC):
            CHl, CHh = CHlo[t], CHhi[t]
            CHt = CHl + CHh
            cb = int(cumCH[t])
            S_sb = spool.tile([P, CHt, P], BF16)
            nc.sync.dma_start(S_sb[:], S_in[:, ds(cb * P, CHt * P)].rearrange(
                "p (c e) -> p c e", e=P))
            S2_sb = spool.tile([P, CHt, P], BF16)
            nc.sync.dma_start(S2_sb[:],
                              S2_in[:, ds(cb * P, CHt * P)].rearrange(
                                  "p (c e) -> p c e", e=P))
            adst = epool.tile([P, F_D], BF16)
            nc.gpsimd.indirect_dma_start(
                out=adst[:], out_offset=None, in_=Tb[:],
                in_offset=bass.IndirectOffsetOnAxis(
                    ap=dn_sb[:, t:t + 1], axis=0))
            G_t = epool.tile([P, CHt, TW], BF16)
            nc.gpsimd.dma_gather(
                G_t[:, 0:CHl, :], T_lo[:, :],
                il_sb[:, ds(8 * int(cumlo[t]), 8 * CHl)],
                P * CHl, P * CHl, TW, single_packet=False)
            nc.gpsimd.dma_gather(
                G_t[:, CHl:CHt, :], T_hi[:, :],
                ih_sb[:, ds(8 * int(cumhi[t]), 8 * CHh)],
                P * CHh, P * CHh, TW, single_packet=False)

            # per-edge a_dst via S2 matmuls (one per chunk)
            psL = ppl.tile([P, CHt, F_D], F32)
            for c in range(CHt):
                nc.tensor.matmul(psL[:, c, :], S2_sb[:, c, :], adst[:],
                                 start=True, stop=True)
            # w = exp(leakyrelu(a_src + a_dst))
            L = epool.tile([P, CHt, F_D], F32)
            nc.vector.tensor_add(L[:], G_t[:, :, F_H:F_G], psL[:])
            Lr = epool.tile([P, CHt, F_D], F32)
            nc.scalar.activation(Lr[:], L[:],
                                 mybir.ActivationFunctionType.Lrelu,
                                 alpha=NEG_SLOPE)
            rhs = epool.tile([P, CHt, RW], BF16)
            nc.scalar.activation(rhs[:, :, F_H:RW], Lr[:],
                                 mybir.ActivationFunctionType.Exp)
            for h in range(F_D):
                nc.vector.tensor_mul(
                    rhs[:, :, h * CW:(h + 1) * CW],
                    G_t[:, :, h * CW:(h + 1) * CW],
                    rhs[:, :, F_H + h:F_H + h + 1].to_broadcast(
                        (P, CHt, CW)))
            # aggregate into PSUM via the one-hot S matmuls
            psE = ppe.tile([P, RW], F32)
            for c in range(CHt):
                nc.tensor.matmul(psE[:], S_sb[:, c, :], rhs[:, c, :],
                                 start=(c == 0), stop=(c == CHt - 1))

            # ---- postprocess this dst-tile ----
            if layer == 1:
                den = opool.tile([P, F_D], F32)
                nc.vector.tensor_scalar_add(den[:], psE[:, F_H:RW], 1e-16)
                rec = opool.tile([P, F_D], F32)
                nc.vector.reciprocal(rec[:], den[:])
                o1 = opool.tile([P, F_H], F32)
                for h in range(F_D):
                    nc.vector.tensor_mul(
                        o1[:, h * CW:(h + 1) * CW],
                        psE[:, h * CW:(h + 1) * CW],
                        rec[:, h:h + 1].to_broadcast((P, CW)))
                nc.vector.tensor_add(o1[:], o1[:], bias_sb[:])
                # elu(x) = max(x,0) + exp(min(x,0)) - 1
                mn = opool.tile([P, F_H], F32)
                nc.vector.tensor_scalar_min(mn[:], o1[:], 0.0)
                em = opool.tile([P, F_H], F32)
                nc.scalar.activation(em[:], mn[:],
                                     mybir.ActivationFunctionType.Exp)
                mx = opool.tile([P, F_H], F32)
                nc.vector.tensor_scalar_max(mx[:], o1[:], 0.0)
                s1 = opool.tile([P, F_H], F32)
                nc.vector.tensor_add(s1[:], mx[:], em[:])
                ob = opool.tile([P, F_H], BF16)
                nc.vector.tensor_scalar_add(ob[:], s1[:], -1.0)
                nc.sync.dma_start(out_dram[ts(t, P), :], ob[:])
            else:
                den = opool.tile([P, 1], F32)
                nc.vector.tensor_scalar_add(den[:], psE[:, F_H:RW], 1e-16)
                rec = opool.tile([P, 1], F32)
                nc.vector.reciprocal(rec[:], den[:])
                o2 = opool.tile([P, F_H], F32)
                nc.vector.tensor_mul(
                    o2[:], psE[:, 0:F_H], rec[:].to_broadcast((P, F_H)))
                nc.vector.tensor_add(o2[:], o2[:], bias_sb[:])
                rm = opool.tile([P, 1], F32)
                nc.vector.tensor_reduce(rm[:], o2[:], mybir.AxisListType.X,
                                        mybir.AluOpType.max)
                z = opool.tile([P, F_H], F32)
                nc.vector.tensor_tensor(z[:], o2[:],
                                        rm[:].to_broadcast((P, F_H)),
                                        op=mybir.AluOpType.subtract)
                e_t = opool.tile([P, F_H], F32)
                s_t = opool.tile([P, 1], F32)
                nc.scalar.activation(e_t[:], z[:],
                                     mybir.ActivationFunctionType.Exp,
                                     accum_out=s_t[:])
                ls = opool.tile([P, 1], F32)
                nc.scalar.activation(ls[:], s_t[:],
                                     mybir.ActivationFunctionType.Ln)
                of = opool.tile([P, F_H], F32)
                nc.vector.tensor_tensor(of[:], z[:],
                                        ls[:].to_broadcast((P, F_H)),
                                        op=mybir.AluOpType.subtract)
                nc.sync.dma_start(out_dram[ts(t, P), :], of[:])

    nc.compile()
    return nc


def _fold_weights1(W1, att_src1, att_dst1):
    A1s = np.zeros((64, 8), np.float32)
    A1s[np.arange(64), np.arange(64) // 8] = att_src1.reshape(64)
    A1d = np.zeros((64, 8), np.float32)
    A1d[np.arange(64), np.arange(64) // 8] = att_dst1.reshape(64)
    return np.concatenate([W1, W1 @ A1s, W1 @ A1d], axis=1)  # [256, 80]


def _dummy_row():
    row = np.zeros((1, TW), np.float32)
    row[0, 64:72] = -300.0
    return row.astype(NPBF16)


def kernel(x, edge_index, W1, att_src1, att_dst1, bias1,
           W2, att_src2, att_dst2, bias2):
    (idxlo, idxhi, S_arr, S2_arr, dstnode,
     CHlo, CHhi, cumlo, cumhi, cumCH) = _prep_edges(edge_index)
    dummyrow = _dummy_row()

    # ---------- layer 1 ----------
    Wc1 = _fold_weights1(W1, att_src1, att_dst1).astype(NPBF16)
    xT = np.zeros((256, NPAD), NPBF16)
    xT[:, :N] = x.T.astype(NPBF16)

    nc1 = _build_layer_program(256, 8, CHlo, CHhi, cumlo, cumhi, cumCH,
                               layer=1)
    in_maps = [{
        "xT": xT, "wc": Wc1, "dummyrow": dummyrow,
        "idxlo": idxlo[k], "idxhi": idxhi[k],
        "S": S_arr[k], "S2": S2_arr[k],
        "dstnode": np.ascontiguousarray(dstnode[k]),
        "bias": bias1.astype(np.float32).reshape(1, 64),
    } for k in range(NCORES)]
    res1 = run_bass_kernel_spmd(nc1, in_maps, core_ids=list(range(NCORES)))
    PERF["layer1_ns"] = res1.exec_time_ns
    PERF["layer1_profile"] = res1.profile_json
    x2 = np.concatenate([res1.results[k]["out"] for k in range(NCORES)],
                        axis=0)  # [NPAD, 64] bf16

    # ---------- layer 2 ----------
    Wc2 = np.concatenate(
        [W2, W2 @ att_src2.T, W2 @ att_dst2.T], axis=1).astype(NPBF16)
    x2T = np.ascontiguousarray(x2.T)  # [64, NPAD] bf16

    nc2 = _build_layer_program(64, 1, CHlo, CHhi, cumlo, cumhi, cumCH,
                               layer=2)
    in_maps2 = [{
        "xT": x2T, "wc": Wc2, "dummyrow": dummyrow,
        "idxlo": idxlo[k], "idxhi": idxhi[k],
        "S": S_arr[k], "S2": S2_arr[k],
        "dstnode": np.ascontiguousarray(dstnode[k]),
        "bias": bias2.astype(np.float32).reshape(1, 64),
    } for k in range(NCORES)]
    res2 = run_bass_kernel_spmd(nc2, in_maps2, core_ids=list(range(NCORES)))
    PERF["layer2_ns"] = res2.exec_time_ns
    PERF["layer2_profile"] = res2.profile_json
    out = np.concatenate([res2.results[k]["out"] for k in range(NCORES)],
                         axis=0)  # [NPAD, 64] f32
    return out[:N].astype(np.float32)
